# revision 19
# baseline (speedup 1.0000x reference)
"""Multi-head GAT layer (PyG-style) as a Trainium2 Bass kernel, 8-way SPMD.

v3 design (dst-sharded, 4-queue ucode gathers, drop-recover rows):
  - Nodes sharded by dst across 8 cores (6250 each, padded to 6272).
  - Phase 1 (projection, sharded 8x): each core projects only its shard with
    a column-PERMUTED weight matrix; row = per head [31 kept xp channels |
    a_j] = 256 f16 = 512B. The dropped channel (largest |att_j| coeff per
    head) is linearly recoverable from the aggregated a_j slot. a_i goes to
    a separate local table. AllGather builds the full 50176-row table.
  - Phase 2 (edge pass): edges grouped by dst block and table half (int16
    gather indices), 128-edge tiles. Per (block, half) one ucode dma_gather
    (512B rows), round-robin over 4 SWDGE queues (each queue runs on its own
    Q7 pair -> 4 gathers in flight).
  - a_i per edge: one-hot S built by tensor_scalar is_equal (per-partition
    scalar), DMA-transposed (HWDGE xbar, scalar/sync engines) to S^T, then
    a PE matmul S^T @ ai_blk expands a_i to edges. No PE transposes, no
    PSUM round-trips.
  - ex = exp(lrelu(a_i + a_j)) (unnormalized softmax, eps-exact); xw = row *
    ex (the a_j slot becomes the recovery carrier); one matmul per tile
    scatters [xw | ex] into PSUM [acc | den]. After the block accumulates,
    the dropped channels are recovered in-place and the block is finalized
    (normalize, LayerNorm, ELU, residual; all in permuted channel order,
    un-permuted on the host).
"""

import numpy as np

# ---- problem constants (hardcoded per spec) ----
N_NODES = 50000
N_EDGES = 800000
IN_CH = 256
HEADS = 8
HEAD_DIM = 32
HC = HEADS * HEAD_DIM  # 256
NEG_SLOPE = 0.2
LN_EPS = 1e-5
SOFTMAX_EPS = 1e-16
M_CORES = 8

P = 128
RW = 256                            # table row: per head [31 kept | aj]
PROJ_W = 264                        # proj cols: 256 row + 8 ai


def _ceil_div(a, b):
    return (a + b - 1) // b


class Plan:
    """Host-side preprocessing: shapes + per-core arrays."""

    def __init__(self, x, edge_index, lin_w, att, ln_w, ln_b,
                 n_nodes=None):
        self.n_cores = M_CORES
        N = x.shape[0] if n_nodes is None else n_nodes
        self.n_nodes = N
        self.shard = N // M_CORES
        self.nb = _ceil_div(self.shard, P)
        self.shard_pad = self.nb * P
        self.tbl = self.shard_pad * M_CORES
        self.half = self.tbl // 2
        assert self.half <= 32767 and self.tbl - self.half <= 32767
        SH, NBL, HALF = self.shard, self.nb, self.half

        src = np.asarray(edge_index[0], dtype=np.int64)
        dst = np.asarray(edge_index[1], dtype=np.int64)

        lw = np.asarray(lin_w, dtype=np.float32)
        at = np.asarray(att, dtype=np.float32)
        lw3 = lw.reshape(IN_CH, HEADS, HEAD_DIM)
        b_i = np.einsum("chk,hk->ch", lw3, at[:, :HEAD_DIM])
        b_j = np.einsum("chk,hk->ch", lw3, at[:, HEAD_DIM:])
        attj = at[:, HEAD_DIM:]  # [H, 32]

        # drop-recover permutation: per head drop argmax|attj| channel
        perm = np.zeros(HC, np.int64)     # slot -> original channel
        vmat = np.zeros(HC, np.float32)   # recovery coefficients
        is_aj = np.zeros(HC, bool)
        for h in range(HEADS):
            ch = int(np.argmax(np.abs(attj[h])))
            kept = [c for c in range(HEAD_DIM) if c != ch]
            inv = 1.0 / attj[h, ch]
            for j, c in enumerate(kept):
                perm[h * 32 + j] = h * 32 + c
                vmat[h * 32 + j] = -attj[h, c] * inv
            perm[h * 32 + 31] = h * 32 + ch   # recovered slot
            vmat[h * 32 + 31] = inv
            is_aj[h * 32 + 31] = True
        self.perm = perm
        self.use_lnw = not np.allclose(np.asarray(ln_w), 1.0)
        self.use_lnb = not np.allclose(np.asarray(ln_b), 0.0)

        # projection weights in slot order; aj slots get b_j columns
        w_cols = np.zeros((IN_CH, PROJ_W), np.float32)
        for s in range(HC):
            if is_aj[s]:
                w_cols[:, s] = b_j[:, s // 32]
            else:
                w_cols[:, s] = lw[:, perm[s]]
        w_cols[:, HC:] = b_i
        w_ext_f16 = w_cols.astype(np.float16)

        xf = np.asarray(x, dtype=np.float32)

        # per-core edge partition, grouped by (dst block, table half)
        per_core = []
        cnt_lo = np.zeros((M_CORES, NBL), np.int64)
        cnt_hi = np.zeros((M_CORES, NBL), np.int64)
        for c in range(M_CORES):
            sel = (dst // SH) == c
            s_c = src[sel]
            d_c = dst[sel] - c * SH
            rows = (s_c // SH) * self.shard_pad + (s_c % SH)
            grp = (rows >= HALF).astype(np.int64)
            blk = d_c // P
            order = np.lexsort((grp, blk))
            rows, d_c, blk, grp = (a[order] for a in (rows, d_c, blk, grp))
            per_core.append((rows, d_c, blk, grp))
            for b in range(NBL):
                m = blk == b
                cnt_lo[c, b] = int(np.sum(m & (grp == 0)))
                cnt_hi[c, b] = int(np.sum(m & (grp == 1)))
        self.t_lo = int(_ceil_div(int(cnt_lo.max()), P))
        self.t_hi = int(_ceil_div(int(cnt_hi.max()), P))
        self.t_tot = self.t_lo + self.t_hi
        t_lo, t_hi, t_tot = self.t_lo, self.t_hi, self.t_tot

        lnw_mat = np.ascontiguousarray(np.broadcast_to(
            np.asarray(ln_w, np.float32)[perm], (P, HC)))
        lnb_mat = np.ascontiguousarray(np.broadcast_to(
            np.asarray(ln_b, np.float32)[perm], (P, HC)))
        vmat_b = np.ascontiguousarray(
            np.broadcast_to(vmat, (P, HC)))

        self.in_maps = []
        for c in range(M_CORES):
            rows, d_c, blk, grp = per_core[c]
            idx16 = np.zeros((16, 8 * t_tot * NBL), np.int16)
            dstc = np.full((P, t_tot * NBL), P, np.float16)
            dst_flat = np.full((1, t_tot * NBL * P), P, np.float16)
            eye129 = np.zeros((P + 1, P), np.float16)
            eye129[:P] = np.eye(P, dtype=np.float16)
            s_rows = np.zeros((t_tot * NBL, P, P), np.float16)
            for b in range(NBL):
                m = blk == b
                for g, toff, tcnt in ((0, 0, t_lo), (1, t_lo, t_hi)):
                    if tcnt == 0:
                        continue
                    mg = m & (grp == g)
                    rel = rows[mg] - (HALF if g else 0)
                    dl = d_c[mg]
                    n = rel.shape[0]
                    cap = tcnt * P
                    relp = np.zeros(cap, np.int64)
                    relp[:n] = rel
                    dlp = np.full(cap, P, np.int64)
                    dlp[:n] = dl % P
                    gcol0 = 8 * (b * t_tot + toff)
                    idx16[:, gcol0:gcol0 + 8 * tcnt] = (
                        relp.astype(np.int16).reshape(-1, 16).T)
                    dstc[:, b * t_tot + toff:b * t_tot + toff + tcnt] = (
                        dlp.astype(np.float16).reshape(-1, P).T)
                    fl0 = (b * t_tot + toff) * P
                    dst_flat[0, fl0:fl0 + cap] = dlp.astype(np.float16)
                    s_rows[b * t_tot + toff:b * t_tot + toff + tcnt] = (
                        eye129[dlp].reshape(tcnt, P, P))
            idx_full = np.tile(idx16, (8, 1))

            xs = np.zeros((IN_CH, self.shard_pad), np.float16)
            xs[:, :SH] = xf[c * SH:(c + 1) * SH].T.astype(np.float16)

            x_res = np.zeros((self.shard_pad, HC), np.float32)
            x_res[:SH] = xf[c * SH:(c + 1) * SH][:, perm] - 1.0

            self.in_maps.append({
                "xT": xs,
                "w_ext": w_ext_f16,
                "idx": idx_full,
                "dstc": dstc,
                "dst_flat": dst_flat,
                "s_rows": np.ascontiguousarray(
                    s_rows.transpose(1, 0, 2).reshape(P, t_tot * NBL * P)),
                "x_res": x_res,
                "lnw_mat": lnw_mat,
                "lnb_mat": lnb_mat,
                "vmat": vmat_b,
            })

    def cache_key(self):
        return (self.t_lo, self.t_hi, self.n_nodes,
                self.use_lnw, self.use_lnb)


def build_nc(plan, probe=None):
    import concourse.bass as bass
    import concourse.bacc as bacc
    import concourse.mybir as mybir
    import concourse.tile as tile
    from concourse import library_config

    fp16 = mybir.dt.float16
    fp32 = mybir.dt.float32
    i16 = mybir.dt.int16
    Alu = mybir.AluOpType
    Act = mybir.ActivationFunctionType

    T_LO, T_HI, T = plan.t_lo, plan.t_hi, plan.t_tot
    NBL, SHARD_PAD, TBL, HALF = plan.nb, plan.shard_pad, plan.tbl, plan.half

    nc = bacc.Bacc(None, target_bir_lowering=False, debug=False,
                   num_devices=M_CORES, num_swdge_queues=4)

    xT = nc.dram_tensor("xT", [IN_CH, SHARD_PAD], fp16, kind="ExternalInput")
    w_ext = nc.dram_tensor("w_ext", [IN_CH, PROJ_W], fp16,
                           kind="ExternalInput")
    idx = nc.dram_tensor("idx", [P, 8 * T * NBL], i16, kind="ExternalInput")
    dstc = nc.dram_tensor("dstc", [P, T * NBL], fp16, kind="ExternalInput")
    dst_flat = nc.dram_tensor("dst_flat", [1, T * NBL * P], fp16,
                              kind="ExternalInput")
    s_rows = nc.dram_tensor("s_rows", [P, T * NBL * P], fp16,
                            kind="ExternalInput")
    x_res = nc.dram_tensor("x_res", [SHARD_PAD, HC], fp32,
                           kind="ExternalInput")
    lnw_mat = nc.dram_tensor("lnw_mat", [P, HC], fp32, kind="ExternalInput")
    lnb_mat = nc.dram_tensor("lnb_mat", [P, HC], fp32, kind="ExternalInput")
    vmat = nc.dram_tensor("vmat", [P, HC], fp32, kind="ExternalInput")
    out = nc.dram_tensor("out", [SHARD_PAD, HC], fp32, kind="ExternalOutput")

    tbl_shard = nc.dram_tensor("tbl_shard", [SHARD_PAD, RW], fp16)
    table = nc.dram_tensor("table", [TBL, RW], fp16, addr_space="Shared")
    ai_tbl = nc.dram_tensor("ai_tbl", [SHARD_PAD, 8], fp16)

    iota_row_np = np.broadcast_to(
        np.arange(P, dtype=np.float16), (P, P)).copy()
    iota_col_np = np.arange(P, dtype=np.float16).reshape(P, 1).copy()
    ident_np = np.eye(P, dtype=np.float16)
    use_lnw = getattr(plan, "use_lnw", True)
    use_lnb = getattr(plan, "use_lnb", True)

    with tile.TileContext(nc) as tc:
        iota_row_dr = nc.inline_tensor(iota_row_np, name="iota_row")
        iota_col_dr = nc.inline_tensor(iota_col_np, name="iota_col")
        ident_dr = nc.inline_tensor(ident_np, name="ident")

        with tc.tile_pool(name="const", bufs=1) as cpool:
            iota_row = cpool.tile([P, P], fp16)
            nc.sync.dma_start(iota_row[:], iota_row_dr[:])
            wk = cpool.tile([P, 2, PROJ_W], fp16)
            nc.sync.dma_start(wk[:], w_ext[:].rearrange("(k p) w -> p k w",
                                                        p=P))
            lnw = cpool.tile([P, HC], fp32)
            nc.sync.dma_start(lnw[:], lnw_mat[:])
            lnb = cpool.tile([P, HC], fp32)
            nc.sync.dma_start(lnb[:], lnb_mat[:])
            vm = cpool.tile([P, HC], fp32)
            nc.sync.dma_start(vm[:], vmat[:])
            eps_t = cpool.tile([P, 1], fp32)
            nc.vector.memset(eps_t[:], LN_EPS)
            idx_sb = cpool.tile([P, 8 * T * NBL], i16)
            nc.sync.dma_start(idx_sb[:], idx[:])
            dst_sb = cpool.tile([P, T * NBL], fp16)
            nc.sync.dma_start(dst_sb[:], dstc[:])
            ones_row = cpool.tile([1, P], fp16)
            nc.vector.memset(ones_row[:], 1.0)
            iota_col = cpool.tile([P, 1], fp16)
            nc.sync.dma_start(iota_col[:], iota_col_dr[:])
            ident = cpool.tile([P, P], fp16)
            nc.sync.dma_start(ident[:], ident_dr[:])

            nc.gpsimd.load_library(library_config.mlp)

            # ---- phase 1: sharded projection ----
            with tc.tile_pool(name="psum_p", bufs=4, space="PSUM") as psp, \
                 tc.tile_pool(name="sb_xall", bufs=1) as sbx, \
                 tc.tile_pool(name="sb_proj", bufs=2) as sbp:
                xall = sbx.tile([P, 2, SHARD_PAD], fp16)
                nc.sync.dma_start(xall[:, 0, :], xT[0:P, :])
                nc.sync.dma_start(xall[:, 1, :], xT[P:2 * P, :])
                for i in range(NBL):
                    pp = psp.tile([P, PROJ_W], fp32, tag="pp")
                    nc.tensor.matmul(pp[:], lhsT=xall[:, 0, i * P:(i + 1) * P],
                                     rhs=wk[:, 0, :], start=True, stop=False)
                    nc.tensor.matmul(pp[:], lhsT=xall[:, 1, i * P:(i + 1) * P],
                                     rhs=wk[:, 1, :], start=False, stop=True)
                    xpa = sbp.tile([P, PROJ_W], fp16, tag="xpa")
                    nc.scalar.copy(xpa[:], pp[:])
                    rows = slice(i * P, (i + 1) * P)
                    nc.sync.dma_start(tbl_shard[rows, :], xpa[:, 0:RW])
                    nc.sync.dma_start(ai_tbl[rows, :], xpa[:, RW:PROJ_W])

            # ---- all-gather the table across the 8 cores ----
            nc.gpsimd.collective_compute(
                "AllGather", Alu.bypass,
                replica_groups=[list(range(M_CORES))],
                ins=[tbl_shard[:]],
                outs=[table[:]],
            )

            if probe is not None and probe.startswith("table"):
                k = int(probe[5:])
                with tc.tile_pool(name="pr", bufs=2) as pr:
                    for b in range(NBL):
                        tf = pr.tile([P, HC], fp16, tag="tf")
                        nc.sync.dma_start(
                            tf[:], table[k * SHARD_PAD + b * P:
                                         k * SHARD_PAD + (b + 1) * P, 0:HC])
                        tg = pr.tile([P, HC], fp32, tag="tg")
                        nc.vector.tensor_copy(tg[:], tf[:])
                        nc.sync.dma_start(out[b * P:(b + 1) * P, :], tg[:])

            run_edge = probe in (None, "gather", "ai")
            # ---- phase 2: edge pass ----
            with tc.tile_pool(name="sb_edge", bufs=2) as sbe, \
                 tc.tile_pool(name="sb_gat", bufs=3) as sbg, \
                 tc.tile_pool(name="sb_fin", bufs=2) as sbf, \
                 tc.tile_pool(name="sb_bat", bufs=1) as sbb, \
                 tc.tile_pool(name="ps_acc", bufs=3, space="PSUM") as psa, \
                 tc.tile_pool(name="ps_bc", bufs=3, space="PSUM") as psb, \
                 tc.tile_pool(name="ps_ai", bufs=2, space="PSUM") as psai:
                acc_all = sbb.tile([P, NBL, RW + 8], fp32)
                for b in range(NBL if run_edge else 0):
                    nrow0 = b * P
                    ai_blk = sbe.tile([P, 8], fp16, tag="ai_blk")
                    nc.sync.dma_start(ai_blk[:], ai_tbl[nrow0:nrow0 + P, :])
                    xg = sbg.tile([P, T, RW], fp16, tag="xg")
                    for g, toff, tcnt in ((0, 0, T_LO), (1, T_LO, T_HI)):
                        if tcnt == 0:
                            continue
                        src_ap = (table[0:HALF, :] if g == 0
                                  else table[HALF:TBL, :])
                        gcol0 = 8 * (b * T + toff)
                        nc.gpsimd.dma_gather(
                            out_ap=xg[:, toff:toff + tcnt, :],
                            in_ap=src_ap,
                            idxs_ap=idx_sb[:, gcol0:gcol0 + 8 * tcnt],
                            num_idxs=tcnt * P,
                            num_idxs_reg=tcnt * P,
                            elem_size=RW,
                            single_packet=False,
                            queue_num=(b * 2 + g) % 4,
                        )
                    if probe == "gather":
                        tg = sbf.tile([P, HC], fp32, tag="tg")
                        nc.vector.tensor_copy(tg[:], xg[:, 0, 0:HC])
                        nc.sync.dma_start(out[nrow0:nrow0 + P, :], tg[:])
                        continue
                    # one-hot dst matrices: S (edge-major) via block TT,
                    # S^T via rank-1 broadcast matmul + block TT
                    s_all = sbe.tile([P, T, P], fp16, tag="s_all")
                    nc.sync.dma_start(
                        s_all[:], s_rows[:, b * T * P:(b + 1) * T * P])
                    st_all = sbe.tile([P, T, P], fp16, tag="st_all")
                    ai_ps = psai.tile([P, T, 8], fp32, tag="ai_ps")
                    for t in range(T):
                        st_ps = psb.tile([P, P], fp16, tag="st_ps")
                        nc.tensor.transpose(out=st_ps[:], in_=s_all[:, t, :],
                                            identity=ident[:])
                        nc.scalar.copy(st_all[:, t, :], st_ps[:])
                        nc.tensor.matmul(ai_ps[:, t, :],
                                         lhsT=st_all[:, t, :],
                                         rhs=ai_blk[:],
                                         start=True, stop=True)
                    # alpha / lrelu / exp
                    aj_view = xg[:].rearrange("p t (h c) -> p t h c",
                                              c=HEAD_DIM)[:, :, :, 31:32]
                    al = sbe.tile([P, T, 8], fp32, tag="al")
                    nc.vector.tensor_tensor(
                        out=al[:],
                        in0=ai_ps[:],
                        in1=aj_view.rearrange("p t h one -> p t (h one)"),
                        op=Alu.add)
                    nc.vector.scalar_tensor_tensor(
                        out=al[:], in0=al[:], scalar=NEG_SLOPE, in1=al[:],
                        op0=Alu.mult, op1=Alu.max)
                    xwex = sbe.tile([P, T, RW + 8], fp16, tag="xwex")
                    nc.scalar.activation(xwex[:, :, RW:RW + 8], al[:],
                                         Act.Exp)
                    nc.vector.tensor_tensor(
                        out=xwex[:, :, 0:RW].rearrange(
                            "p t (h c) -> p t h c", h=HEADS),
                        in0=xg[:].rearrange("p t (h c) -> p t h c", h=HEADS),
                        in1=xwex[:, :, RW:RW + 8].to_broadcast(
                            [P, T, HEADS, HEAD_DIM]),
                        op=Alu.mult)
                    # scatter-accumulate [acc | den]
                    accden = psa.tile([P, RW + 8], fp32, tag="accden")
                    for t in range(T):
                        nc.tensor.matmul(accden[:], lhsT=s_all[:, t, :],
                                         rhs=xwex[:, t, :],
                                         start=(t == 0), stop=(t == T - 1))
                    # recover dropped channels in-place (PSUM), then stash
                    tmp = sbf.tile([P, HC], fp32, tag="tmp")
                    nc.vector.tensor_tensor(out=tmp[:], in0=accden[:, 0:HC],
                                            in1=vm[:], op=Alu.mult)
                    drop8 = sbf.tile([P, 8], fp32, tag="drop8")
                    nc.vector.tensor_reduce(
                        out=drop8[:],
                        in_=tmp[:].rearrange("p (h c) -> p h c", h=HEADS),
                        axis=mybir.AxisListType.X, op=Alu.add)
                    nc.vector.tensor_copy(
                        accden[:, 0:HC].rearrange(
                            "p (h c) -> p h c", h=HEADS)[:, :, 31:32]
                        .rearrange("p h one -> p (h one)"),
                        drop8[:])
                    nc.scalar.copy(acc_all[:, b, :], accden[:])

                if run_edge:
                    AV = acc_all[:, :, 0:HC]
                    d8a = sbb.tile([P, NBL, 8], fp32)
                    nc.vector.tensor_scalar_add(
                        d8a[:], acc_all[:, :, HC:HC + 8], SOFTMAX_EPS)
                    r8a = sbb.tile([P, NBL, 8], fp32)
                    nc.vector.reciprocal(r8a[:], d8a[:])
                    nc.vector.tensor_tensor(
                        out=AV.rearrange("p b (h c) -> p b h c", h=HEADS),
                        in0=AV.rearrange("p b (h c) -> p b h c", h=HEADS),
                        in1=r8a[:].to_broadcast([P, NBL, HEADS, HEAD_DIM]),
                        op=Alu.mult)
                    st6a = sbb.tile([P, NBL, 6], fp32)
                    mva = sbb.tile([P, NBL, 2], fp32)
                    for b in range(NBL):
                        nc.vector.bn_stats(st6a[:, b, :],
                                           acc_all[:, b, 0:HC])
                        nc.vector.bn_aggr(mva[:, b, :], st6a[:, b, :])
                    negmu = sbb.tile([P, NBL], fp32)
                    nc.vector.tensor_scalar_mul(
                        negmu[:], mva[:, :, 0:1].rearrange(
                            "p b one -> p (b one)"), -1.0)
                    sdva = sbb.tile([P, NBL], fp32)
                    nc.scalar.activation(
                        sdva[:], mva[:, :, 1:2].rearrange(
                            "p b one -> p (b one)"),
                        Act.Sqrt, bias=eps_t[:, 0:1], scale=1.0)
                    rstda = sbb.tile([P, NBL], fp32)
                    nc.vector.reciprocal(rstda[:], sdva[:])
                    nc.vector.tensor_tensor(
                        out=AV, in0=AV,
                        in1=negmu[:].to_broadcast([P, NBL, HC]),
                        op=Alu.add)
                    nc.vector.tensor_tensor(
                        out=AV, in0=AV,
                        in1=rstda[:].to_broadcast([P, NBL, HC]),
                        op=Alu.mult)
                    if use_lnw:
                        lnw_bc = bass.AP(lnw[:].tensor, lnw[:].offset,
                                         [lnw[:].ap[0], [0, NBL],
                                          lnw[:].ap[1]])
                        nc.vector.tensor_tensor(out=AV, in0=AV, in1=lnw_bc,
                                                op=Alu.mult)
                    if use_lnb:
                        lnb_bc = bass.AP(lnb[:].tensor, lnb[:].offset,
                                         [lnb[:].ap[0], [0, NBL],
                                          lnb[:].ap[1]])
                        nc.vector.tensor_tensor(out=AV, in0=AV, in1=lnb_bc,
                                                op=Alu.add)
                    # per-block ELU + residual tail (exp-only ACT, no table
                    # switches): elu(y)+x = max(y,0) + min(exp(y),1) + (x-1)
                    for b in range(NBL):
                        nrow0 = b * P
                        yc = acc_all[:, b, 0:HC]
                        ee = sbf.tile([P, HC], fp32, tag="ee")
                        nc.scalar.activation(ee[:], yc, Act.Exp)
                        xr = sbf.tile([P, HC], fp32, tag="xr")
                        nc.sync.dma_start(xr[:], x_res[nrow0:nrow0 + P, :])
                        f1 = sbf.tile([P, HC], fp32, tag="f1")
                        nc.vector.scalar_tensor_tensor(
                            out=f1[:], in0=ee[:], scalar=1.0, in1=xr[:],
                            op0=Alu.min, op1=Alu.add)
                        fin = sbf.tile([P, HC], fp32, tag="fin")
                        nc.vector.scalar_tensor_tensor(
                            out=fin[:], in0=yc, scalar=0.0, in1=f1[:],
                            op0=Alu.max, op1=Alu.add)
                        nc.sync.dma_start(out[nrow0:nrow0 + P, :], fin[:])

    nc.compile()
    return nc


_NC_CACHE = {}


def get_nc(plan, probe=None):
    key = plan.cache_key() + (probe,)
    if key not in _NC_CACHE:
        _NC_CACHE[key] = build_nc(plan, probe=probe)
    return _NC_CACHE[key]


def postprocess(plan, results):
    outs = [res["out"][:plan.shard] for res in results]
    got = np.concatenate(outs, axis=0).astype(np.float32)
    full = np.empty_like(got)
    full[:, plan.perm] = got
    return full


def _run(plan, trace=False, probe=None):
    from concourse.bass_utils import run_bass_kernel_spmd
    nc = get_nc(plan, probe=probe)
    r = run_bass_kernel_spmd(nc, plan.in_maps,
                             core_ids=list(range(plan.n_cores)), trace=trace)
    return postprocess(plan, r.results), r


def kernel(x, edge_index, lin_w, att, ln_w, ln_b):
    plan = Plan(x, edge_index, lin_w, att, ln_w, ln_b)
    out, _ = _run(plan)
    return out


# ---------------- self-contained mini test ----------------
def _np_reference(x, edge_index, lin_w, att, ln_w, ln_b):
    N = x.shape[0]
    src, dst = edge_index[0], edge_index[1]
    xp = (x @ lin_w).reshape(N, HEADS, HEAD_DIM)
    a_i = np.einsum("nhc,hc->nh", xp, att[:, :HEAD_DIM])
    a_j = np.einsum("nhc,hc->nh", xp, att[:, HEAD_DIM:])
    alpha = a_i[dst] + a_j[src]
    alpha = np.where(alpha >= 0, alpha, NEG_SLOPE * alpha)
    amax = np.full((N, HEADS), -np.inf, np.float32)
    np.maximum.at(amax, dst, alpha)
    amax = np.where(np.isfinite(amax), amax, 0.0)
    ex = np.exp(alpha - amax[dst])
    denom = np.zeros((N, HEADS), np.float32)
    np.add.at(denom, dst, ex)
    alpha = ex / (denom[dst] + SOFTMAX_EPS)
    msg = xp[src] * alpha[:, :, None]
    outv = np.zeros((N, HEADS, HEAD_DIM), np.float32)
    np.add.at(outv, dst, msg)
    outv = outv.reshape(N, HC)
    mu = outv.mean(-1, keepdims=True)
    var = ((outv - mu) ** 2).mean(-1, keepdims=True)
    outv = (outv - mu) / np.sqrt(var + LN_EPS) * ln_w + ln_b
    outv = np.where(outv > 0, outv, np.exp(np.minimum(outv, 0)) - 1)
    return outv + x


# revision 20
# speedup vs baseline: 1.0738x; 1.0738x over previous
"""Multi-head GAT layer (PyG-style) as a Trainium2 Bass kernel, 8-way SPMD.

v3 design (dst-sharded, 4-queue ucode gathers, drop-recover rows):
  - Nodes sharded by dst across 8 cores (6250 each, padded to 6272).
  - Phase 1 (projection, sharded 8x): each core projects only its shard with
    a column-PERMUTED weight matrix; row = per head [31 kept xp channels |
    a_j] = 256 f16 = 512B. The dropped channel (largest |att_j| coeff per
    head) is linearly recoverable from the aggregated a_j slot. a_i goes to
    a separate local table. AllGather builds the full 50176-row table.
  - Phase 2 (edge pass): edges grouped by dst block and table half (int16
    gather indices), 128-edge tiles. Per (block, half) one ucode dma_gather
    (512B rows), round-robin over 4 SWDGE queues (each queue runs on its own
    Q7 pair -> 4 gathers in flight).
  - a_i per edge: one-hot S built by tensor_scalar is_equal (per-partition
    scalar), DMA-transposed (HWDGE xbar, scalar/sync engines) to S^T, then
    a PE matmul S^T @ ai_blk expands a_i to edges. No PE transposes, no
    PSUM round-trips.
  - ex = exp(lrelu(a_i + a_j)) (unnormalized softmax, eps-exact); xw = row *
    ex (the a_j slot becomes the recovery carrier); one matmul per tile
    scatters [xw | ex] into PSUM [acc | den]. After the block accumulates,
    the dropped channels are recovered in-place and the block is finalized
    (normalize, LayerNorm, ELU, residual; all in permuted channel order,
    un-permuted on the host).
"""

import numpy as np

# ---- problem constants (hardcoded per spec) ----
N_NODES = 50000
N_EDGES = 800000
IN_CH = 256
HEADS = 8
HEAD_DIM = 32
HC = HEADS * HEAD_DIM  # 256
NEG_SLOPE = 0.2
LN_EPS = 1e-5
SOFTMAX_EPS = 1e-16
M_CORES = 8

P = 128
RW = 256                            # table row: per head [31 kept | aj]
PROJ_W = 264                        # proj cols: 256 row + 8 ai


def _ceil_div(a, b):
    return (a + b - 1) // b


class Plan:
    """Host-side preprocessing: shapes + per-core arrays."""

    def __init__(self, x, edge_index, lin_w, att, ln_w, ln_b,
                 n_nodes=None):
        self.n_cores = M_CORES
        N = x.shape[0] if n_nodes is None else n_nodes
        self.n_nodes = N
        self.shard = N // M_CORES
        self.nb = _ceil_div(self.shard, P)
        self.shard_pad = self.nb * P
        self.tbl = self.shard_pad * M_CORES
        self.half = self.tbl // 2
        assert self.half <= 32767 and self.tbl - self.half <= 32767
        SH, NBL, HALF = self.shard, self.nb, self.half

        src = np.asarray(edge_index[0], dtype=np.int64)
        dst = np.asarray(edge_index[1], dtype=np.int64)

        lw = np.asarray(lin_w, dtype=np.float32)
        at = np.asarray(att, dtype=np.float32)
        lw3 = lw.reshape(IN_CH, HEADS, HEAD_DIM)
        b_i = np.einsum("chk,hk->ch", lw3, at[:, :HEAD_DIM])
        b_j = np.einsum("chk,hk->ch", lw3, at[:, HEAD_DIM:])
        attj = at[:, HEAD_DIM:]  # [H, 32]

        # drop-recover permutation: per head drop argmax|attj| channel
        perm = np.zeros(HC, np.int64)     # slot -> original channel
        vmat = np.zeros(HC, np.float32)   # recovery coefficients
        is_aj = np.zeros(HC, bool)
        for h in range(HEADS):
            ch = int(np.argmax(np.abs(attj[h])))
            kept = [c for c in range(HEAD_DIM) if c != ch]
            inv = 1.0 / attj[h, ch]
            for j, c in enumerate(kept):
                perm[h * 32 + j] = h * 32 + c
                vmat[h * 32 + j] = -attj[h, c] * inv
            perm[h * 32 + 31] = h * 32 + ch   # recovered slot
            vmat[h * 32 + 31] = inv
            is_aj[h * 32 + 31] = True
        self.perm = perm
        self.use_lnw = not np.allclose(np.asarray(ln_w), 1.0)
        self.use_lnb = not np.allclose(np.asarray(ln_b), 0.0)

        # projection weights in slot order; aj slots get b_j columns
        w_cols = np.zeros((IN_CH, PROJ_W), np.float32)
        for s in range(HC):
            if is_aj[s]:
                w_cols[:, s] = b_j[:, s // 32]
            else:
                w_cols[:, s] = lw[:, perm[s]]
        w_cols[:, HC:] = b_i
        w_ext_f16 = w_cols.astype(np.float16)

        xf = np.asarray(x, dtype=np.float32)

        # per-core edge partition, grouped by (dst block, table half)
        per_core = []
        cnt_lo = np.zeros((M_CORES, NBL), np.int64)
        cnt_hi = np.zeros((M_CORES, NBL), np.int64)
        for c in range(M_CORES):
            sel = (dst // SH) == c
            s_c = src[sel]
            d_c = dst[sel] - c * SH
            rows = (s_c // SH) * self.shard_pad + (s_c % SH)
            grp = (rows >= HALF).astype(np.int64)
            blk = d_c // P
            order = np.lexsort((grp, blk))
            rows, d_c, blk, grp = (a[order] for a in (rows, d_c, blk, grp))
            per_core.append((rows, d_c, blk, grp))
            for b in range(NBL):
                m = blk == b
                cnt_lo[c, b] = int(np.sum(m & (grp == 0)))
                cnt_hi[c, b] = int(np.sum(m & (grp == 1)))
        self.t_lo = int(_ceil_div(int(cnt_lo.max()), P))
        self.t_hi = int(_ceil_div(int(cnt_hi.max()), P))
        self.t_tot = self.t_lo + self.t_hi
        t_lo, t_hi, t_tot = self.t_lo, self.t_hi, self.t_tot

        lnw_mat = np.ascontiguousarray(np.broadcast_to(
            np.asarray(ln_w, np.float32)[perm], (P, HC)))
        lnb_mat = np.ascontiguousarray(np.broadcast_to(
            np.asarray(ln_b, np.float32)[perm], (P, HC)))
        vmat_b = np.ascontiguousarray(
            np.broadcast_to(vmat, (P, HC)))

        self.in_maps = []
        for c in range(M_CORES):
            rows, d_c, blk, grp = per_core[c]
            idx16 = np.zeros((16, 8 * t_tot * NBL), np.int16)
            dstc = np.full((P, t_tot * NBL), P, np.float16)
            dst_flat = np.full((1, t_tot * NBL * P), P, np.float16)
            eye129 = np.zeros((P + 1, P), np.float16)
            eye129[:P] = np.eye(P, dtype=np.float16)
            s_rows = np.zeros((t_tot * NBL, P, P), np.float16)
            for b in range(NBL):
                m = blk == b
                for g, toff, tcnt in ((0, 0, t_lo), (1, t_lo, t_hi)):
                    if tcnt == 0:
                        continue
                    mg = m & (grp == g)
                    rel = rows[mg] - (HALF if g else 0)
                    dl = d_c[mg]
                    n = rel.shape[0]
                    cap = tcnt * P
                    relp = np.zeros(cap, np.int64)
                    relp[:n] = rel
                    dlp = np.full(cap, P, np.int64)
                    dlp[:n] = dl % P
                    gcol0 = 8 * (b * t_tot + toff)
                    idx16[:, gcol0:gcol0 + 8 * tcnt] = (
                        relp.astype(np.int16).reshape(-1, 16).T)
                    dstc[:, b * t_tot + toff:b * t_tot + toff + tcnt] = (
                        dlp.astype(np.float16).reshape(-1, P).T)
                    fl0 = (b * t_tot + toff) * P
                    dst_flat[0, fl0:fl0 + cap] = dlp.astype(np.float16)
                    s_rows[b * t_tot + toff:b * t_tot + toff + tcnt] = (
                        eye129[dlp].reshape(tcnt, P, P))
            idx_full = np.tile(idx16, (8, 1))

            xs = np.zeros((IN_CH, self.shard_pad), np.float16)
            xs[:, :SH] = xf[c * SH:(c + 1) * SH].T.astype(np.float16)

            x_res = np.zeros((self.shard_pad, HC), np.float32)
            x_res[:SH] = xf[c * SH:(c + 1) * SH][:, perm] - 1.0

            self.in_maps.append({
                "xT": xs,
                "w_ext": w_ext_f16,
                "idx": idx_full,
                "dstc": dstc,
                "dst_flat": dst_flat,
                "s_rows": np.ascontiguousarray(
                    s_rows.transpose(1, 0, 2).reshape(P, t_tot * NBL * P)),
                "x_res": x_res,
                "lnw_mat": lnw_mat,
                "lnb_mat": lnb_mat,
                "vmat": vmat_b,
            })

    def cache_key(self):
        return (self.t_lo, self.t_hi, self.n_nodes,
                self.use_lnw, self.use_lnb)


def build_nc(plan, probe=None):
    import concourse.bass as bass
    import concourse.bacc as bacc
    import concourse.mybir as mybir
    import concourse.tile as tile
    from concourse import library_config

    fp16 = mybir.dt.float16
    fp32 = mybir.dt.float32
    i16 = mybir.dt.int16
    Alu = mybir.AluOpType
    Act = mybir.ActivationFunctionType

    T_LO, T_HI, T = plan.t_lo, plan.t_hi, plan.t_tot
    NBL, SHARD_PAD, TBL, HALF = plan.nb, plan.shard_pad, plan.tbl, plan.half

    nc = bacc.Bacc(None, target_bir_lowering=False, debug=False,
                   num_devices=M_CORES, num_swdge_queues=4)

    xT = nc.dram_tensor("xT", [IN_CH, SHARD_PAD], fp16, kind="ExternalInput")
    w_ext = nc.dram_tensor("w_ext", [IN_CH, PROJ_W], fp16,
                           kind="ExternalInput")
    idx = nc.dram_tensor("idx", [P, 8 * T * NBL], i16, kind="ExternalInput")
    dstc = nc.dram_tensor("dstc", [P, T * NBL], fp16, kind="ExternalInput")
    dst_flat = nc.dram_tensor("dst_flat", [1, T * NBL * P], fp16,
                              kind="ExternalInput")
    s_rows = nc.dram_tensor("s_rows", [P, T * NBL * P], fp16,
                            kind="ExternalInput")
    x_res = nc.dram_tensor("x_res", [SHARD_PAD, HC], fp32,
                           kind="ExternalInput")
    lnw_mat = nc.dram_tensor("lnw_mat", [P, HC], fp32, kind="ExternalInput")
    lnb_mat = nc.dram_tensor("lnb_mat", [P, HC], fp32, kind="ExternalInput")
    vmat = nc.dram_tensor("vmat", [P, HC], fp32, kind="ExternalInput")
    out = nc.dram_tensor("out", [SHARD_PAD, HC], fp32, kind="ExternalOutput")

    tbl_shard = nc.dram_tensor("tbl_shard", [SHARD_PAD, RW], fp16)
    table = nc.dram_tensor("table", [TBL, RW], fp16, addr_space="Shared")
    ai_tbl = nc.dram_tensor("ai_tbl", [SHARD_PAD, 8], fp16)

    iota_row_np = np.broadcast_to(
        np.arange(P, dtype=np.float16), (P, P)).copy()
    iota_col_np = np.arange(P, dtype=np.float16).reshape(P, 1).copy()
    ident_np = np.eye(P, dtype=np.float16)
    use_lnw = getattr(plan, "use_lnw", True)
    use_lnb = getattr(plan, "use_lnb", True)

    with tile.TileContext(nc) as tc:
        iota_row_dr = nc.inline_tensor(iota_row_np, name="iota_row")
        iota_col_dr = nc.inline_tensor(iota_col_np, name="iota_col")
        ident_dr = nc.inline_tensor(ident_np, name="ident")

        with tc.tile_pool(name="const", bufs=1) as cpool:
            iota_row = cpool.tile([P, P], fp16)
            nc.sync.dma_start(iota_row[:], iota_row_dr[:])
            wk = cpool.tile([P, 2, PROJ_W], fp16)
            nc.sync.dma_start(wk[:], w_ext[:].rearrange("(k p) w -> p k w",
                                                        p=P))
            lnw = cpool.tile([P, HC], fp32)
            nc.sync.dma_start(lnw[:], lnw_mat[:])
            lnb = cpool.tile([P, HC], fp32)
            nc.sync.dma_start(lnb[:], lnb_mat[:])
            vm = cpool.tile([P, HC], fp32)
            nc.sync.dma_start(vm[:], vmat[:])
            eps_t = cpool.tile([P, 1], fp32)
            nc.vector.memset(eps_t[:], LN_EPS)
            idx_sb = cpool.tile([P, 8 * T * NBL], i16)
            nc.sync.dma_start(idx_sb[:], idx[:])
            dst_sb = cpool.tile([P, T * NBL], fp16)
            nc.sync.dma_start(dst_sb[:], dstc[:])
            ones_row = cpool.tile([1, P], fp16)
            nc.vector.memset(ones_row[:], 1.0)
            iota_col = cpool.tile([P, 1], fp16)
            nc.sync.dma_start(iota_col[:], iota_col_dr[:])
            ident = cpool.tile([P, P], fp16)
            nc.sync.dma_start(ident[:], ident_dr[:])

            nc.gpsimd.load_library(library_config.mlp)

            # ---- phase 1: sharded projection ----
            with tc.tile_pool(name="psum_p", bufs=4, space="PSUM") as psp, \
                 tc.tile_pool(name="sb_xall", bufs=1) as sbx, \
                 tc.tile_pool(name="sb_proj", bufs=2) as sbp:
                xall = sbx.tile([P, 2, SHARD_PAD], fp16)
                nc.sync.dma_start(xall[:, 0, :], xT[0:P, :])
                nc.sync.dma_start(xall[:, 1, :], xT[P:2 * P, :])
                for i in range(NBL):
                    pp = psp.tile([P, PROJ_W], fp32, tag="pp")
                    nc.tensor.matmul(pp[:], lhsT=xall[:, 0, i * P:(i + 1) * P],
                                     rhs=wk[:, 0, :], start=True, stop=False)
                    nc.tensor.matmul(pp[:], lhsT=xall[:, 1, i * P:(i + 1) * P],
                                     rhs=wk[:, 1, :], start=False, stop=True)
                    xpa = sbp.tile([P, PROJ_W], fp16, tag="xpa")
                    nc.scalar.copy(xpa[:], pp[:])
                    rows = slice(i * P, (i + 1) * P)
                    nc.sync.dma_start(tbl_shard[rows, :], xpa[:, 0:RW])
                    nc.sync.dma_start(ai_tbl[rows, :], xpa[:, RW:PROJ_W])

            # ---- all-gather the table across the 8 cores ----
            nc.gpsimd.collective_compute(
                "AllGather", Alu.bypass,
                replica_groups=[list(range(M_CORES))],
                ins=[tbl_shard[:]],
                outs=[table[:]],
            )

            if probe is not None and probe.startswith("table"):
                k = int(probe[5:])
                with tc.tile_pool(name="pr", bufs=2) as pr:
                    for b in range(NBL):
                        tf = pr.tile([P, HC], fp16, tag="tf")
                        nc.sync.dma_start(
                            tf[:], table[k * SHARD_PAD + b * P:
                                         k * SHARD_PAD + (b + 1) * P, 0:HC])
                        tg = pr.tile([P, HC], fp32, tag="tg")
                        nc.vector.tensor_copy(tg[:], tf[:])
                        nc.sync.dma_start(out[b * P:(b + 1) * P, :], tg[:])

            run_edge = probe in (None, "gather", "ai")
            # ---- phase 2: edge pass ----
            with tc.tile_pool(name="sb_edge", bufs=3) as sbe, \
                 tc.tile_pool(name="sb_gat", bufs=3) as sbg, \
                 tc.tile_pool(name="sb_fin", bufs=2) as sbf, \
                 tc.tile_pool(name="sb_bat", bufs=1) as sbb, \
                 tc.tile_pool(name="ps_acc", bufs=3, space="PSUM") as psa, \
                 tc.tile_pool(name="ps_bc", bufs=2, space="PSUM") as psb, \
                 tc.tile_pool(name="ps_ai", bufs=3, space="PSUM") as psai:
                acc_all = sbb.tile([P, NBL, RW + 8], fp32)
                for b in range(NBL if run_edge else 0):
                    nrow0 = b * P
                    ai_blk = sbe.tile([P, 8], fp16, tag="ai_blk")
                    nc.sync.dma_start(ai_blk[:], ai_tbl[nrow0:nrow0 + P, :])
                    xg = sbg.tile([P, T, RW], fp16, tag="xg")
                    for g, toff, tcnt in ((0, 0, T_LO), (1, T_LO, T_HI)):
                        if tcnt == 0:
                            continue
                        src_ap = (table[0:HALF, :] if g == 0
                                  else table[HALF:TBL, :])
                        gcol0 = 8 * (b * T + toff)
                        nc.gpsimd.dma_gather(
                            out_ap=xg[:, toff:toff + tcnt, :],
                            in_ap=src_ap,
                            idxs_ap=idx_sb[:, gcol0:gcol0 + 8 * tcnt],
                            num_idxs=tcnt * P,
                            num_idxs_reg=tcnt * P,
                            elem_size=RW,
                            single_packet=False,
                            queue_num=(b * 2 + g) % 4,
                        )
                    if probe == "gather":
                        tg = sbf.tile([P, HC], fp32, tag="tg")
                        nc.vector.tensor_copy(tg[:], xg[:, 0, 0:HC])
                        nc.sync.dma_start(out[nrow0:nrow0 + P, :], tg[:])
                        continue
                    # one-hot dst matrices: S (edge-major) via block TT,
                    # S^T via rank-1 broadcast matmul + block TT
                    s_all = sbe.tile([P, T, P], fp16, tag="s_all")
                    nc.sync.dma_start(
                        s_all[:], s_rows[:, b * T * P:(b + 1) * T * P])
                    st_all = sbe.tile([P, T, P], fp16, tag="st_all")
                    ai_ps = psai.tile([P, T, 8], fp32, tag="ai_ps")
                    for t in range(T):
                        st_ps = psb.tile([P, P], fp16, tag="st_ps")
                        nc.tensor.transpose(out=st_ps[:], in_=s_all[:, t, :],
                                            identity=ident[:])
                        nc.scalar.copy(st_all[:, t, :], st_ps[:])
                        nc.tensor.matmul(ai_ps[:, t, :],
                                         lhsT=st_all[:, t, :],
                                         rhs=ai_blk[:],
                                         start=True, stop=True)
                    # alpha / lrelu / exp
                    aj_view = xg[:].rearrange("p t (h c) -> p t h c",
                                              c=HEAD_DIM)[:, :, :, 31:32]
                    al = sbe.tile([P, T, 8], fp32, tag="al")
                    nc.vector.tensor_tensor(
                        out=al[:],
                        in0=ai_ps[:],
                        in1=aj_view.rearrange("p t h one -> p t (h one)"),
                        op=Alu.add)
                    nc.vector.scalar_tensor_tensor(
                        out=al[:], in0=al[:], scalar=NEG_SLOPE, in1=al[:],
                        op0=Alu.mult, op1=Alu.max)
                    xwex = sbe.tile([P, T, RW + 8], fp16, tag="xwex")
                    nc.scalar.activation(xwex[:, :, RW:RW + 8], al[:],
                                         Act.Exp)
                    nc.vector.tensor_tensor(
                        out=xwex[:, :, 0:RW].rearrange(
                            "p t (h c) -> p t h c", h=HEADS),
                        in0=xg[:].rearrange("p t (h c) -> p t h c", h=HEADS),
                        in1=xwex[:, :, RW:RW + 8].to_broadcast(
                            [P, T, HEADS, HEAD_DIM]),
                        op=Alu.mult)
                    # scatter-accumulate [acc | den]
                    accden = psa.tile([P, RW + 8], fp32, tag="accden")
                    for t in range(T):
                        nc.tensor.matmul(accden[:], lhsT=s_all[:, t, :],
                                         rhs=xwex[:, t, :],
                                         start=(t == 0), stop=(t == T - 1))
                    # recover dropped channels in-place (PSUM), then stash
                    tmp = sbf.tile([P, HC], fp32, tag="tmp")
                    nc.vector.tensor_tensor(out=tmp[:], in0=accden[:, 0:HC],
                                            in1=vm[:], op=Alu.mult)
                    drop8 = sbf.tile([P, 8], fp32, tag="drop8")
                    nc.vector.tensor_reduce(
                        out=drop8[:],
                        in_=tmp[:].rearrange("p (h c) -> p h c", h=HEADS),
                        axis=mybir.AxisListType.X, op=Alu.add)
                    nc.vector.tensor_copy(
                        accden[:, 0:HC].rearrange(
                            "p (h c) -> p h c", h=HEADS)[:, :, 31:32]
                        .rearrange("p h one -> p (h one)"),
                        drop8[:])
                    nc.scalar.copy(acc_all[:, b, :], accden[:])

                if run_edge:
                    AV = acc_all[:, :, 0:HC]
                    d8a = sbb.tile([P, NBL, 8], fp32)
                    nc.vector.tensor_scalar_add(
                        d8a[:], acc_all[:, :, HC:HC + 8], SOFTMAX_EPS)
                    r8a = sbb.tile([P, NBL, 8], fp32)
                    nc.vector.reciprocal(r8a[:], d8a[:])
                    nc.vector.tensor_tensor(
                        out=AV.rearrange("p b (h c) -> p b h c", h=HEADS),
                        in0=AV.rearrange("p b (h c) -> p b h c", h=HEADS),
                        in1=r8a[:].to_broadcast([P, NBL, HEADS, HEAD_DIM]),
                        op=Alu.mult)
                    st6a = sbb.tile([P, NBL, 6], fp32)
                    mva = sbb.tile([P, NBL, 2], fp32)
                    for b in range(NBL):
                        nc.vector.bn_stats(st6a[:, b, :],
                                           acc_all[:, b, 0:HC])
                        nc.vector.bn_aggr(mva[:, b, :], st6a[:, b, :])
                    negmu = sbb.tile([P, NBL], fp32)
                    nc.vector.tensor_scalar_mul(
                        negmu[:], mva[:, :, 0:1].rearrange(
                            "p b one -> p (b one)"), -1.0)
                    sdva = sbb.tile([P, NBL], fp32)
                    nc.scalar.activation(
                        sdva[:], mva[:, :, 1:2].rearrange(
                            "p b one -> p (b one)"),
                        Act.Sqrt, bias=eps_t[:, 0:1], scale=1.0)
                    rstda = sbb.tile([P, NBL], fp32)
                    nc.vector.reciprocal(rstda[:], sdva[:])
                    nc.vector.tensor_tensor(
                        out=AV, in0=AV,
                        in1=negmu[:].to_broadcast([P, NBL, HC]),
                        op=Alu.add)
                    nc.vector.tensor_tensor(
                        out=AV, in0=AV,
                        in1=rstda[:].to_broadcast([P, NBL, HC]),
                        op=Alu.mult)
                    if use_lnw:
                        lnw_bc = bass.AP(lnw[:].tensor, lnw[:].offset,
                                         [lnw[:].ap[0], [0, NBL],
                                          lnw[:].ap[1]])
                        nc.vector.tensor_tensor(out=AV, in0=AV, in1=lnw_bc,
                                                op=Alu.mult)
                    if use_lnb:
                        lnb_bc = bass.AP(lnb[:].tensor, lnb[:].offset,
                                         [lnb[:].ap[0], [0, NBL],
                                          lnb[:].ap[1]])
                        nc.vector.tensor_tensor(out=AV, in0=AV, in1=lnb_bc,
                                                op=Alu.add)
                    # per-block ELU + residual tail (exp-only ACT, no table
                    # switches): elu(y)+x = max(y,0) + min(exp(y),1) + (x-1)
                    for b in range(NBL):
                        nrow0 = b * P
                        yc = acc_all[:, b, 0:HC]
                        ee = sbf.tile([P, HC], fp32, tag="ee")
                        nc.scalar.activation(ee[:], yc, Act.Exp)
                        xr = sbf.tile([P, HC], fp32, tag="xr")
                        nc.sync.dma_start(xr[:], x_res[nrow0:nrow0 + P, :])
                        f1 = sbf.tile([P, HC], fp32, tag="f1")
                        nc.vector.scalar_tensor_tensor(
                            out=f1[:], in0=ee[:], scalar=1.0, in1=xr[:],
                            op0=Alu.min, op1=Alu.add)
                        fin = sbf.tile([P, HC], fp32, tag="fin")
                        nc.vector.scalar_tensor_tensor(
                            out=fin[:], in0=yc, scalar=0.0, in1=f1[:],
                            op0=Alu.max, op1=Alu.add)
                        nc.sync.dma_start(out[nrow0:nrow0 + P, :], fin[:])

    nc.compile()
    return nc


_NC_CACHE = {}


def get_nc(plan, probe=None):
    key = plan.cache_key() + (probe,)
    if key not in _NC_CACHE:
        _NC_CACHE[key] = build_nc(plan, probe=probe)
    return _NC_CACHE[key]


def postprocess(plan, results):
    outs = [res["out"][:plan.shard] for res in results]
    got = np.concatenate(outs, axis=0).astype(np.float32)
    full = np.empty_like(got)
    full[:, plan.perm] = got
    return full


def _run(plan, trace=False, probe=None):
    from concourse.bass_utils import run_bass_kernel_spmd
    nc = get_nc(plan, probe=probe)
    r = run_bass_kernel_spmd(nc, plan.in_maps,
                             core_ids=list(range(plan.n_cores)), trace=trace)
    return postprocess(plan, r.results), r


def kernel(x, edge_index, lin_w, att, ln_w, ln_b):
    plan = Plan(x, edge_index, lin_w, att, ln_w, ln_b)
    out, _ = _run(plan)
    return out


# ---------------- self-contained mini test ----------------
def _np_reference(x, edge_index, lin_w, att, ln_w, ln_b):
    N = x.shape[0]
    src, dst = edge_index[0], edge_index[1]
    xp = (x @ lin_w).reshape(N, HEADS, HEAD_DIM)
    a_i = np.einsum("nhc,hc->nh", xp, att[:, :HEAD_DIM])
    a_j = np.einsum("nhc,hc->nh", xp, att[:, HEAD_DIM:])
    alpha = a_i[dst] + a_j[src]
    alpha = np.where(alpha >= 0, alpha, NEG_SLOPE * alpha)
    amax = np.full((N, HEADS), -np.inf, np.float32)
    np.maximum.at(amax, dst, alpha)
    amax = np.where(np.isfinite(amax), amax, 0.0)
    ex = np.exp(alpha - amax[dst])
    denom = np.zeros((N, HEADS), np.float32)
    np.add.at(denom, dst, ex)
    alpha = ex / (denom[dst] + SOFTMAX_EPS)
    msg = xp[src] * alpha[:, :, None]
    outv = np.zeros((N, HEADS, HEAD_DIM), np.float32)
    np.add.at(outv, dst, msg)
    outv = outv.reshape(N, HC)
    mu = outv.mean(-1, keepdims=True)
    var = ((outv - mu) ** 2).mean(-1, keepdims=True)
    outv = (outv - mu) / np.sqrt(var + LN_EPS) * ln_w + ln_b
    outv = np.where(outv > 0, outv, np.exp(np.minimum(outv, 0)) - 1)
    return outv + x
